# Initial kernel scaffold
#
"""DenseCRF mean-field inference on 8 Trainium2 NeuronCores.

Strategy (sharding_hint: shard N=H*W pixels across cores):
  - Each core owns a 1152-pixel block (12 image rows) = one column-block
    K[:, blk] of the symmetric N x N combined kernel
    K = 10*K_bi + 3*K_sm  (bilateral + smoothness Gaussians).
  - Build phase (on device): d2 via small matmuls over float16 hi/lo-split
    augmented features (exact to ~1e-4), ScalarE exp with the weights folded
    into the bias, scaled by 4096.  Stored as
        Khi = bf16(4096*K)           (SBUF-resident, 162 KB/partition)
        Klo = fp8_e4m3(4096*K - Khi) (HBM, streamed back each iteration)
    giving ~12-13 bit kernel mantissa; measured end-to-end max-rel error
    vs the f32 reference is ~2.6e-3 (at the f32 noise floor).
  - 5 mean-field iterations: per-core matmul (contraction over all N pixels,
    128-chunk accumulation in PSUM, col-tiled 2x for the bf16 hi pass with
    hi/lo-split marginals as weights [M=42] and 4x for the fp8 lo pass
    [M=21]), PSUM partials folded+descaled by one constant matmul,
    Potts compatibility folded in analytically (softmax is shift-invariant:
    (pw @ (ones-eye)).T update == subtracting pw, so we keep
    nunary = -unary and ADD), PE-transpose + per-128-pixel-tile softmax,
    AllGather of the (1152,21) marginals per iteration.
  - Output: each core writes its (1152, 21) block; host concatenates.
"""

import math
import numpy as np

C = 21
H = W = 96
N = H * W                  # 9216
NCORES = 8
NB = N // NCORES           # 1152 pixels per core
NCHUNK = N // 128          # 72 contraction chunks
JS = [(0, 512), (512, 512), (1024, 128)]   # jsub splits of the 1152 block
ITERS = 5
SCALE = 4096.0
APP_W, SMO_W = 10.0, 3.0
PROX_STD, COLOR_STD, SMOOTH_STD = 30.0, 0.1, 3.0
DBI, DSM = 19, 10          # augmented feature dims (bilateral / smoothness)
DF = DBI + DSM             # packed feature rows

_COMPILED = {}


def _split16(v):
    """f32 vector -> (hi, lo) float16 pair, hi+lo == v to ~2^-22."""
    hi = v.astype(np.float16)
    lo = (v - hi.astype(np.float32)).astype(np.float16)
    return hi, lo


def _aug_features(f):
    """f: (N, D) f32 -> A (Daug, N), B (Daug, N) float16 so that
    (A[:, i] . B[:, j]) accumulated in f32 == ||f_i - f_j||^2 to ~1e-4."""
    n, d = f.shape
    fc = (f - f.mean(axis=0, keepdims=True)).astype(np.float32)
    a16, b16 = _split16(fc.reshape(-1))
    a16 = a16.reshape(n, d); b16 = b16.reshape(n, d)
    sq = np.sum(fc.astype(np.float64) ** 2, axis=1).astype(np.float32)
    s1, s2 = _split16(sq)
    ones = np.ones(n, np.float16)
    A = np.concatenate(
        [a16.T, a16.T, b16.T, s1[None], s2[None], ones[None], ones[None]], axis=0
    )
    Bm = np.concatenate(
        [-2.0 * a16.astype(np.float32).T, -2.0 * b16.astype(np.float32).T,
         -2.0 * a16.astype(np.float32).T,
         np.ones((2, n), np.float32), s1[None].astype(np.float32),
         s2[None].astype(np.float32)], axis=0
    ).astype(np.float16)
    return A.astype(np.float16), Bm


def _build_nc():
    import concourse.bass as bass
    import concourse.mybir as mybir
    import concourse.tile as tile

    f32 = mybir.dt.float32
    f16 = mybir.dt.float16
    bf16 = mybir.dt.bfloat16
    f8 = mybir.dt.float8e4
    Exp = mybir.ActivationFunctionType.Exp
    add = mybir.AluOpType.add
    sub = mybir.AluOpType.subtract
    mult = mybir.AluOpType.mult
    amin = mybir.AluOpType.min
    AX = mybir.AxisListType.X

    nc = bass.Bass()

    af = nc.declare_dram_parameter("af", [DF, N], f16, isOutput=False)
    bf = nc.declare_dram_parameter("bf", [DF, NB], f16, isOutput=False)
    nyhat = nc.declare_dram_parameter("nyhat", [C, NB], f32, isOutput=False)
    ident = nc.declare_dram_parameter("ident", [C, C], f32, isOutput=False)
    foldA = nc.declare_dram_parameter("foldA", [128, C], f32, isOutput=False)
    foldL = nc.declare_dram_parameter("foldL", [128, C], f32, isOutput=False)
    out_ext = nc.declare_dram_parameter("out_blk", [NB, C], f32, isOutput=True)

    LN_BI = float(math.log(APP_W * SCALE))
    LN_SM = float(math.log(SMO_W * SCALE))

    with tile.TileContext(nc) as tc:
        with (
            tc.tile_pool(name="big", bufs=1) as big,
            tc.tile_pool(name="sb", bufs=3) as sb,
            tc.tile_pool(name="sm", bufs=4) as sm,
            tc.tile_pool(name="dram", bufs=1, space="DRAM") as dram,
            tc.tile_pool(name="agd", bufs=2, space="DRAM") as agd,
            tc.tile_pool(name="ptp", bufs=2, space="PSUM") as ptp,
        ):
            # ---------------- persistent SBUF tensors ----------------
            khi = big.tile([128, NCHUNK, NB], bf16, tag="khi")     # 162 KB/part
            w3 = big.tile([128, NCHUNK, 2 * C], bf16, tag="w3")    # hi|lo weights
            wf8 = big.tile([128, NCHUNK, C], f8, tag="wf8")
            nunary = big.tile([C, NB], f32, tag="nunary")
            ident_sb = big.tile([C, C], f32, tag="ident")
            foldA_sb = big.tile([128, C], f32, tag="foldA")
            foldL_sb = big.tile([128, C], f32, tag="foldL")
            bf_sb = big.tile([DF, NB], f16, tag="bf")

            klo_dram = dram.tile([NCHUNK, 128, NB], f8)

            nc.sync.dma_start(ident_sb[:], ident[:])
            nc.sync.dma_start(foldA_sb[:], foldA[:])
            nc.sync.dma_start(foldL_sb[:], foldL[:])
            nc.sync.dma_start(bf_sb[:], bf[:])
            nc.sync.dma_start(nunary[:], nyhat[:])

            # ---------------- build: K column-block ----------------
            with tc.tile_pool(name="pbld", bufs=2, space="PSUM") as pbld:
                for g in range(NCHUNK):
                    afc = sb.tile([DF, 128], f16, tag="afc")
                    nc.sync.dma_start(afc[:], af[:, g * 128:(g + 1) * 128])
                    for j0, jw in JS:
                        d2b = pbld.tile([128, 512], f32, tag="d2b")
                        d2s = pbld.tile([128, 512], f32, tag="d2s")
                        nc.tensor.matmul(
                            d2b[:, :jw], afc[:DBI, :], bf_sb[:DBI, j0:j0 + jw],
                            start=True, stop=True)
                        nc.tensor.matmul(
                            d2s[:, :jw], afc[DBI:, :], bf_sb[DBI:, j0:j0 + jw],
                            start=True, stop=True)
                        t1 = sb.tile([128, 512], f32, tag="t1")
                        t2 = sb.tile([128, 512], f32, tag="t2")
                        nc.scalar.activation(t1[:, :jw], d2b[:, :jw], Exp,
                                             bias=LN_BI, scale=-0.5)
                        nc.scalar.activation(t2[:, :jw], d2s[:, :jw], Exp,
                                             bias=LN_SM, scale=-0.5)
                        s = sb.tile([128, 512], f32, tag="s")
                        nc.vector.tensor_tensor(s[:, :jw], t1[:, :jw], t2[:, :jw], add)
                        ksl = khi[:, g, j0:j0 + jw]
                        nc.gpsimd.tensor_copy(ksl, s[:, :jw])
                        kl = sb.tile([128, 512], f8, tag="kl")
                        nc.vector.tensor_tensor(kl[:, :jw], s[:, :jw], ksl, sub)
                        nc.sync.dma_start(klo_dram[g, :, j0:j0 + jw], kl[:, :jw])

            # ---------------- iterations ----------------
            def softmax_phase(final):
                """nunary (C,1152) -> per-128-pixel softmax -> agin / out_ext."""
                agin = None
                if not final:
                    agin = agd.tile([NB, C], f32, tag="agin")
                for h in range(9):
                    tp = ptp.tile([128, C], f32, tag="tp")
                    nc.tensor.transpose(
                        tp[:], nunary[:, h * 128:(h + 1) * 128], ident_sb[:])
                    mn = sm.tile([128, 1], f32, tag="mn")
                    nc.vector.tensor_reduce(mn[:], tp[:], AX, amin)
                    e = sm.tile([128, C], f32, tag="e")
                    ssum = sm.tile([128, 1], f32, tag="ssum")
                    nc.scalar.activation(e[:], tp[:], Exp, bias=mn[:], scale=-1.0,
                                         accum_out=ssum[:])
                    r = sm.tile([128, 1], f32, tag="r")
                    nc.vector.reciprocal(r[:], ssum[:])
                    ob = sm.tile([128, C], f32, tag="ob")
                    nc.vector.tensor_scalar_mul(ob[:], e[:], r[:])
                    dst = out_ext if final else agin
                    nc.sync.dma_start(dst[h * 128:(h + 1) * 128, :], ob[:])
                return agin

            def gather_and_weights(agin):
                agout = agd.tile([N, C], f32, tag="agout")
                nc.gpsimd.collective_compute(
                    "AllGather", mybir.AluOpType.bypass,
                    replica_groups=[list(range(NCORES))],
                    ins=[agin.opt()], outs=[agout.opt()])
                ot = sb.tile([128, NCHUNK, C], f32, tag="ot")
                nc.sync.dma_start(
                    ot[:], agout.rearrange("(g p) c -> p g c", p=128))
                nc.vector.tensor_scalar_mul(w3[:, :, 0:C], ot[:], 1.0)
                nc.vector.tensor_tensor(w3[:, :, C:2 * C], ot[:], w3[:, :, 0:C], sub)
                nc.vector.tensor_scalar_mul(wf8[:], ot[:], 1.0)

            # initial marginals from the unaries
            gather_and_weights(softmax_phase(final=False))

            with tc.tile_pool(name="pit", bufs=2, space="PSUM") as pit:
                for it in range(ITERS):
                    for j0, jw in JS:
                        pa = pit.tile([128, 512], f32, tag="pa")
                        pl = pit.tile([128, 512], f32, tag="pl")
                        for g in range(NCHUNK):
                            p2, p4 = g % 2, g % 4
                            nc.tensor.matmul(
                                pa[64 * p2:64 * p2 + 2 * C, :jw],
                                w3[:, g, :], khi[:, g, j0:j0 + jw],
                                start=(g < 2), stop=(g >= NCHUNK - 2),
                                tile_position=(0, 64 * p2))
                            kt = sb.tile([128, 512], f8, tag="klo")
                            nc.sync.dma_start(kt[:, :jw], klo_dram[g, :, j0:j0 + jw])
                            nc.tensor.matmul(
                                pl[32 * p4:32 * p4 + C, :jw],
                                wf8[:, g, :], kt[:, :jw],
                                start=(g < 4), stop=(g >= NCHUNK - 4),
                                tile_position=(0, 32 * p4))
                        # fold col-tiled partials (+1/SCALE descale) via matmul
                        sa = sb.tile([128, 512], f32, tag="sa")
                        sl = sb.tile([128, 512], f32, tag="sl")
                        nc.scalar.copy(sa[:, :jw], pa[:, :jw])
                        nc.scalar.copy(sl[:, :jw], pl[:, :jw])
                        pf = ptp.tile([C, 512], f32, tag="pf")
                        nc.tensor.matmul(pf[:, :jw], foldA_sb[:], sa[:, :jw],
                                         start=True, stop=False)
                        nc.tensor.matmul(pf[:, :jw], foldL_sb[:], sl[:, :jw],
                                         start=False, stop=True)
                        nc.vector.tensor_tensor(
                            nunary[:, j0:j0 + jw], nunary[:, j0:j0 + jw],
                            pf[:, :jw], add)
                    final = it == ITERS - 1
                    agin = softmax_phase(final)
                    if not final:
                        gather_and_weights(agin)

    nc.compile()
    return nc


def _host_prep(x, yhat):
    x = np.asarray(x, np.float32)
    yhat = np.asarray(yhat, np.float32)
    yy, xx = np.mgrid[0:H, 0:W]
    pos = np.stack([yy, xx]).astype(np.float32)
    sig = np.array([PROX_STD, PROX_STD, COLOR_STD, COLOR_STD, COLOR_STD],
                   np.float32)
    f_bi = (np.concatenate([pos, x.reshape(3, H, W)], 0).reshape(5, -1).T
            / sig).astype(np.float32)
    f_sm = (pos.reshape(2, -1).T / SMOOTH_STD).astype(np.float32)
    Abi, Bbi = _aug_features(f_bi)
    Asm, Bsm = _aug_features(f_sm)
    af = np.concatenate([Abi, Asm], axis=0)          # (29, N) f16
    bfull = np.concatenate([Bbi, Bsm], axis=0)       # (29, N) f16
    nyh = (-yhat.reshape(C, N)).astype(np.float32)

    identity = np.eye(C, dtype=np.float32)
    foldA = np.zeros((128, C), np.float32)
    foldL = np.zeros((128, C), np.float32)
    inv = np.float32(1.0 / SCALE)
    for off in (0, C, 64, 64 + C):
        foldA[off:off + C] += identity * inv
    for off in (0, 32, 64, 96):
        foldL[off:off + C] += identity * inv

    in_maps = []
    for b in range(NCORES):
        cols = slice(b * NB, (b + 1) * NB)
        in_maps.append({
            "af": af,
            "bf": np.ascontiguousarray(bfull[:, cols]),
            "nyhat": np.ascontiguousarray(nyh[:, cols]),
            "ident": identity,
            "foldA": foldA,
            "foldL": foldL,
        })
    return in_maps


def kernel(x, yhat, mu):
    from concourse.bass_utils import run_bass_kernel_spmd

    if "nc" not in _COMPILED:
        _COMPILED["nc"] = _build_nc()
    nc = _COMPILED["nc"]

    in_maps = _host_prep(x, yhat)
    res = run_bass_kernel_spmd(nc, in_maps, list(range(NCORES)))
    blocks = [res.results[b]["out_blk"] for b in range(NCORES)]
    full = np.concatenate(blocks, axis=0)            # (N, C)
    return np.ascontiguousarray(full.T.reshape(C, H, W).astype(np.float32))


# revision 16
# speedup vs baseline: 1.0197x; 1.0197x over previous
"""DenseCRF mean-field inference on 8 Trainium2 NeuronCores.

Strategy (sharding_hint: shard N=H*W pixels across cores):
  - Each core owns a 1152-pixel block (12 image rows) = one column-block
    K[:, blk] of the symmetric N x N combined kernel
    K = 10*K_bi + 3*K_sm  (bilateral + smoothness Gaussians).
  - Build phase (on device): d2 via small matmuls over float16 hi/lo-split
    augmented features (exact to ~1e-4), ScalarE exp with the weights folded
    into the bias, scaled by 4096.  Stored as
        Khi = bf16(4096*K)           (SBUF-resident, 162 KB/partition)
        Klo = fp8_e4m3(4096*K - Khi) (HBM, streamed back each iteration)
    giving ~12-13 bit kernel mantissa; measured end-to-end max-rel error
    vs the f32 reference is ~2e-3 (at the f32 noise floor).
  - 5 mean-field iterations: per-core matmul (contraction over all N pixels,
    128-chunk accumulation in PSUM, col-tiled 2x with hi/lo-split marginals
    as weights [M=42 bf16] plus an fp8 lo pass [M=21]), PSUM partials
    folded+descaled by constant matmuls, Potts compatibility folded in
    analytically (softmax is shift-invariant: the (pw @ (ones-eye)).T
    update == subtracting pw, so we keep nunary = -unary and ADD),
    PE-transpose + per-128-pixel-tile softmax, AllGather of the (1152,21)
    marginals per iteration.
  - Output: each core writes its (1152, 21) block; host concatenates.
"""

import math
import numpy as np

C = 21
H = W = 96
N = H * W                  # 9216
NCORES = 8
NB = N // NCORES           # 1152 pixels per core
NCHUNK = N // 128          # 72 contraction chunks
JS = [(0, 512), (512, 512), (1024, 128)]   # jsub splits of the 1152 block
GB = 8                     # klo chunk batch (jsub 0/1)
GB2 = 16                   # klo chunk batch (jsub 2)
ITERS = 5
SCALE = 4096.0
APP_W, SMO_W = 10.0, 3.0
PROX_STD, COLOR_STD, SMOOTH_STD = 30.0, 0.1, 3.0
DBI, DSM = 19, 10          # augmented feature dims (bilateral / smoothness)
DSM0 = 64                  # smoothness rows at partition 64 (base-32 operands fail)
DF = DSM0 + DSM            # packed feature rows (19 bi, pad, 10 sm)

_COMPILED = {}


def _split16(v):
    """f32 vector -> (hi, lo) float16 pair, hi+lo == v to ~2^-22."""
    hi = v.astype(np.float16)
    lo = (v - hi.astype(np.float32)).astype(np.float16)
    return hi, lo


def _aug_features(f):
    """f: (N, D) f32 -> A (Daug, N), B (Daug, N) float16 so that
    (A[:, i] . B[:, j]) accumulated in f32 == ||f_i - f_j||^2 to ~1e-4."""
    n, d = f.shape
    fc = (f - f.mean(axis=0, keepdims=True)).astype(np.float32)
    a16, b16 = _split16(fc.reshape(-1))
    a16 = a16.reshape(n, d); b16 = b16.reshape(n, d)
    sq = np.sum(fc.astype(np.float64) ** 2, axis=1).astype(np.float32)
    s1, s2 = _split16(sq)
    ones = np.ones(n, np.float16)
    A = np.concatenate(
        [a16.T, a16.T, b16.T, s1[None], s2[None], ones[None], ones[None]], axis=0
    )
    Bm = np.concatenate(
        [-2.0 * a16.astype(np.float32).T, -2.0 * b16.astype(np.float32).T,
         -2.0 * a16.astype(np.float32).T,
         np.ones((2, n), np.float32), s1[None].astype(np.float32),
         s2[None].astype(np.float32)], axis=0
    ).astype(np.float16)
    return A.astype(np.float16), Bm


def _build_nc():
    import concourse.bacc as bacc
    import concourse.mybir as mybir
    import concourse.tile as tile

    f32 = mybir.dt.float32
    f16 = mybir.dt.float16
    bf16 = mybir.dt.bfloat16
    f8 = mybir.dt.float8e4
    Exp = mybir.ActivationFunctionType.Exp
    add = mybir.AluOpType.add
    sub = mybir.AluOpType.subtract
    amin = mybir.AluOpType.min
    AX = mybir.AxisListType.X

    nc = bacc.Bacc(None, target_bir_lowering=False)

    af = nc.declare_dram_parameter("af", [DF, N], f16, isOutput=False)
    bf = nc.declare_dram_parameter("bf", [DF, NB], f16, isOutput=False)
    nyhat = nc.declare_dram_parameter("nyhat", [C, NB], f32, isOutput=False)
    ident = nc.declare_dram_parameter("ident", [C, C], f32, isOutput=False)
    foldA = nc.declare_dram_parameter("foldA", [128, C], f32, isOutput=False)
    foldL = nc.declare_dram_parameter("foldL", [128, C], f32, isOutput=False)
    out_ext = nc.declare_dram_parameter("out_blk", [NB, C], f32, isOutput=True)

    LN_BI = float(math.log(APP_W * SCALE))
    LN_SM = float(math.log(SMO_W * SCALE))

    with tile.TileContext(nc) as tc:
        with (
            tc.tile_pool(name="big", bufs=1) as big,
            tc.tile_pool(name="sbo", bufs=1) as sbo,
            tc.tile_pool(name="sm", bufs=4) as sm,
            tc.tile_pool(name="dram", bufs=1, space="DRAM") as dram,
            tc.tile_pool(name="agd", bufs=2, space="DRAM") as agd,
            tc.tile_pool(name="ptp", bufs=3, space="PSUM") as ptp,
        ):
            # ---------------- persistent SBUF tensors ----------------
            khi = big.tile([128, NCHUNK, NB], bf16, tag="khi")     # 162 KB/part
            w3 = big.tile([128, NCHUNK, 2 * C], bf16, tag="w3")    # hi|lo weights
            wf8 = big.tile([128, NCHUNK, C], f8, tag="wf8")
            nunary = big.tile([C, NB], f32, tag="nunary")
            ident_sb = big.tile([C, C], f32, tag="ident")
            foldA_sb = big.tile([128, C], f32, tag="foldA")
            foldL_sb = big.tile([128, C], f32, tag="foldL")
            bf_sb = big.tile([DF, NB], f16, tag="bf")

            # klo in HBM, laid out so iteration reads are contiguous
            # per-partition runs: klo01[js, p, g, :] / klo2[p, g, :]
            klo01 = dram.tile([2, 128, NCHUNK, 512], f8)
            klo2 = dram.tile([128, NCHUNK, 128], f8)

            nc.sync.dma_start(ident_sb[:], ident[:])
            nc.sync.dma_start(foldA_sb[:], foldA[:])
            nc.sync.dma_start(foldL_sb[:], foldL[:])
            nc.sync.dma_start(bf_sb[:], bf[:])
            nc.sync.dma_start(nunary[:], nyhat[:])

            bias_bi = big.tile([128, 1], f32, tag="bias_bi")
            bias_sm = big.tile([128, 1], f32, tag="bias_sm")
            nc.vector.memset(bias_bi[:], LN_BI)
            nc.vector.memset(bias_sm[:], LN_SM)

            # ---------------- build: K column-block ----------------
            with (
                tc.tile_pool(name="sbb", bufs=2) as sbb,
                tc.tile_pool(name="sbb3", bufs=3) as sbb3,
                tc.tile_pool(name="pbld", bufs=2, space="PSUM") as pbld,
            ):
                for g in range(NCHUNK):
                    afc = sbb3.tile([DF, 128], f16, tag="afc")
                    nc.sync.dma_start(afc[:], af[:, g * 128:(g + 1) * 128])
                    for js, (j0, jw) in enumerate(JS):
                        d2b = pbld.tile([128, 512], f32, tag="d2b")
                        d2s = pbld.tile([128, 512], f32, tag="d2s")
                        nc.tensor.matmul(
                            d2b[:, :jw], afc[:DBI, :], bf_sb[:DBI, j0:j0 + jw],
                            start=True, stop=True)
                        nc.tensor.matmul(
                            d2s[:, :jw], afc[DSM0:, :], bf_sb[DSM0:, j0:j0 + jw],
                            start=True, stop=True)
                        t1 = sbb.tile([128, 512], f32, tag="t1")
                        t2 = sbb.tile([128, 512], f32, tag="t2")
                        nc.scalar.activation(t1[:, :jw], d2b[:, :jw], Exp,
                                             bias=bias_bi[:], scale=-0.5)
                        nc.scalar.activation(t2[:, :jw], d2s[:, :jw], Exp,
                                             bias=bias_sm[:], scale=-0.5)
                        s = sbb.tile([128, 512], f32, tag="s")
                        nc.vector.tensor_tensor(s[:, :jw], t1[:, :jw],
                                                t2[:, :jw], add)
                        ksl = khi[:, g, j0:j0 + jw]
                        nc.vector.tensor_copy(ksl, s[:, :jw])
                        kl = sbb3.tile([128, 512], f8, tag="kl")
                        nc.vector.tensor_tensor(kl[:, :jw], s[:, :jw], ksl, sub)
                        if js < 2:
                            nc.sync.dma_start(klo01[js, :, g, :], kl[:, :jw])
                        else:
                            nc.sync.dma_start(klo2[:, g, :], kl[:, :jw])

            # ---------------- helper phases ----------------
            def softmax_phase(final):
                """nunary (C,1152) -> per-128-pixel softmax -> agin / out_ext."""
                agin = None
                if not final:
                    agin = agd.tile([NB, C], f32, tag="agin")
                for h in range(9):
                    tp = ptp.tile([128, C], f32, tag="tp")
                    nc.tensor.transpose(
                        tp[:], nunary[:, h * 128:(h + 1) * 128], ident_sb[:])
                    mn = sm.tile([128, 1], f32, tag="mn")
                    nc.vector.tensor_reduce(mn[:], tp[:], AX, amin)
                    e = sm.tile([128, C], f32, tag="e")
                    ssum = sm.tile([128, 1], f32, tag="ssum")
                    nc.scalar.activation(e[:], tp[:], Exp, bias=mn[:], scale=-1.0,
                                         accum_out=ssum[:])
                    r = sm.tile([128, 1], f32, tag="r")
                    nc.vector.reciprocal(r[:], ssum[:])
                    ob = sm.tile([128, C], f32, tag="ob")
                    nc.vector.tensor_scalar_mul(ob[:], e[:], r[:])
                    dst = out_ext if final else agin
                    nc.sync.dma_start(dst[h * 128:(h + 1) * 128, :], ob[:])
                return agin

            def gather_and_weights(agin):
                agout = agd.tile([N, C], f32, tag="agout")
                nc.gpsimd.collective_compute(
                    "AllGather", mybir.AluOpType.bypass,
                    replica_groups=[list(range(NCORES))],
                    ins=[agin.opt()], outs=[agout.opt()])
                ot = sbo.tile([128, NCHUNK, C], f32, tag="ot")
                nc.sync.dma_start(
                    ot[:], agout.rearrange("(g p) c -> p g c", p=128))
                nc.vector.tensor_scalar_mul(w3[:, :, 0:C], ot[:], 1.0)
                nc.vector.tensor_tensor(w3[:, :, C:2 * C], ot[:], w3[:, :, 0:C],
                                        sub)
                nc.vector.tensor_scalar_mul(wf8[:], ot[:], 1.0)

            # initial marginals from the unaries
            gather_and_weights(softmax_phase(final=False))

            # ---------------- iterations ----------------
            with (
                tc.tile_pool(name="sbi", bufs=2) as sbi,
                tc.tile_pool(name="pit", bufs=1, space="PSUM") as pit,
            ):
                for it in range(ITERS):
                    for js, (j0, jw) in enumerate(JS):
                        pa = [pit.tile([128, 512], f32, tag=f"pa{p}",
                                       name=f"pa{p}") for p in range(2)]
                        pl = [pit.tile([128, 512], f32, tag=f"pl{p}",
                                       name=f"pl{p}") for p in range(2)]
                        nbatch = GB if js < 2 else GB2
                        for g0 in range(0, NCHUNK, nbatch):
                            nb = min(nbatch, NCHUNK - g0)
                            if js < 2:
                                kt = sbi.tile([128, GB, 512], f8, tag="kt")
                                nc.sync.dma_start(
                                    kt[:, :nb, :], klo01[js, :, g0:g0 + nb, :])
                            else:
                                kt = sbi.tile([128, GB2, 128], f8, tag="kt2")
                                nc.sync.dma_start(
                                    kt[:, :nb, :], klo2[:, g0:g0 + nb, :])
                            for g in range(g0, g0 + nb):
                                p2 = g % 2
                                nc.tensor.matmul(
                                    pa[p2][64 * p2:64 * p2 + 2 * C, :jw],
                                    w3[:, g, :], khi[:, g, j0:j0 + jw],
                                    start=(g < 2), stop=(g >= NCHUNK - 2),
                                    tile_position=(0, 64 * p2))
                                nc.tensor.matmul(
                                    pl[p2][64 * p2:64 * p2 + C, :jw],
                                    wf8[:, g, :], kt[:, g - g0, :jw],
                                    start=(g < 2), stop=(g >= NCHUNK - 2),
                                    tile_position=(0, 64 * p2))
                        # fold col-tiled partials (+1/SCALE descale) via one
                        # matmul per pass; gaps between partial blocks are
                        # zeroed once (first iteration) so the single
                        # contraction over [0:106) / [0:85) reads no garbage
                        sa = sbi.tile([128, 512], f32, tag="sa")
                        sl = sbi.tile([128, 512], f32, tag="sl")
                        if it == 0:
                            nc.vector.memset(sa[:], 0.0)
                            nc.vector.memset(sl[:], 0.0)
                        nc.scalar.copy(sa[0:2 * C, :jw], pa[0][0:2 * C, :jw])
                        nc.scalar.copy(sa[64:64 + 2 * C, :jw],
                                       pa[1][64:64 + 2 * C, :jw])
                        nc.scalar.copy(sl[0:C, :jw], pl[0][0:C, :jw])
                        nc.scalar.copy(sl[64:64 + C, :jw],
                                       pl[1][64:64 + C, :jw])
                        pfh = ptp.tile([C, 512], f32, tag="tp", name="pfh")
                        nc.tensor.matmul(pfh[:, :jw], foldA_sb[0:64 + 2 * C, :],
                                         sa[0:64 + 2 * C, :jw],
                                         start=True, stop=True)
                        pfl = ptp.tile([C, 512], f32, tag="tp", name="pfl")
                        nc.tensor.matmul(pfl[:, :jw], foldL_sb[0:64 + C, :],
                                         sl[0:64 + C, :jw],
                                         start=True, stop=True)
                        nc.vector.tensor_tensor(
                            nunary[:, j0:j0 + jw], nunary[:, j0:j0 + jw],
                            pfh[:, :jw], add)
                        nc.vector.tensor_tensor(
                            nunary[:, j0:j0 + jw], nunary[:, j0:j0 + jw],
                            pfl[:, :jw], add)
                    final = it == ITERS - 1
                    agin = softmax_phase(final)
                    if not final:
                        gather_and_weights(agin)

    nc.compile()
    return nc


def _host_prep(x, yhat):
    x = np.asarray(x, np.float32)
    yhat = np.asarray(yhat, np.float32)
    yy, xx = np.mgrid[0:H, 0:W]
    pos = np.stack([yy, xx]).astype(np.float32)
    sig = np.array([PROX_STD, PROX_STD, COLOR_STD, COLOR_STD, COLOR_STD],
                   np.float32)
    f_bi = (np.concatenate([pos, x.reshape(3, H, W)], 0).reshape(5, -1).T
            / sig).astype(np.float32)
    f_sm = (pos.reshape(2, -1).T / SMOOTH_STD).astype(np.float32)
    Abi, Bbi = _aug_features(f_bi)
    Asm, Bsm = _aug_features(f_sm)
    pad = np.zeros((DSM0 - DBI, N), np.float16)
    af = np.concatenate([Abi, pad, Asm], axis=0)     # (74, N) f16
    bfull = np.concatenate([Bbi, pad, Bsm], axis=0)  # (74, N) f16
    nyh = (-yhat.reshape(C, N)).astype(np.float32)

    identity = np.eye(C, dtype=np.float32)
    foldA = np.zeros((128, C), np.float32)
    foldL = np.zeros((128, C), np.float32)
    inv = np.float32(1.0 / SCALE)
    for off in (0, C, 64, 64 + C):
        foldA[off:off + C] += identity * inv
    for off in (0, 64):
        foldL[off:off + C] += identity * inv

    in_maps = []
    for b in range(NCORES):
        cols = slice(b * NB, (b + 1) * NB)
        in_maps.append({
            "af": af,
            "bf": np.ascontiguousarray(bfull[:, cols]),
            "nyhat": np.ascontiguousarray(nyh[:, cols]),
            "ident": identity,
            "foldA": foldA,
            "foldL": foldL,
        })
    return in_maps


def kernel(x, yhat, mu):
    from concourse.bass_utils import run_bass_kernel_spmd

    if "nc" not in _COMPILED:
        _COMPILED["nc"] = _build_nc()
    nc = _COMPILED["nc"]

    in_maps = _host_prep(x, yhat)
    res = run_bass_kernel_spmd(nc, in_maps, list(range(NCORES)))
    blocks = [res.results[b]["out_blk"] for b in range(NCORES)]
    full = np.concatenate(blocks, axis=0)            # (N, C)
    return np.ascontiguousarray(full.T.reshape(C, H, W).astype(np.float32))
